# revision 27
# baseline (speedup 1.0000x reference)
"""Bass/Trainium2 kernel for nn_CapLayer (dynamic-routing capsule layer).

Key algebraic identity (holds for ANY x, W — verified against the reference):
the routing logits b start at zero; softmax over the out-caps axis of an
o-constant tensor is uniform (1/NUM_OUT); with uniform c the squashed v is
identical for every out-cap o, which makes delta_b = <pred, v> o-constant as
well, so b stays o-constant through every routing iteration and the softmax
stays uniform forever.  Hence:

    sbar[b, :] = (1/64) * sum_n pred[b, n, :]
               = (1/64) * sum_{s,i} (sum_p u[b,s,p,i]) * W[s,:,i]
    v[b, o, :] = sbar[b,:] * (|sbar| / (1 + |sbar|^2))     for all o.

So the kernel is: a full reduction of x over the per-group spatial axis
(memory bound — must read all of x exactly once at ~2.9us/MB on the single
shared DMA-engine resource), a tiny matmul with a rearranged W, a squash,
and a broadcast store.  Data-parallel over batch across 8 cores.

On-chip dataflow per core (8 batches, 8 channel-blocks of 128):
  - DMA chunks [128c, nb, 256m] of x stream in back-to-back (~23.3us total).
  - DVE: segmented reduce over the 8 spatial repeats: [128c, nb, 256] ->
    tk[128c, nb, 32i].
  - PE (A2): per (batch, group-of-4): lhsT=tk[:, b, :], rhs=sel[:, g] ->
    u3[j][32g:32g+32, b] in PSUM, i.e. u3 lands directly in the stacked
    [(g i), b] layout the B-stage needs.
  - DVE: one [128, 8] f32->bf16 copy per block -> u3sb[j].
  - PE (B): ONE bf16 matmul per block: sbar[8b, 64o] += u3sb[j]^T @ wt3[j]
    (wt3[(g i), j, o] = W[4j+g, o, i]/64, pre-scaled+packed bf16 on host).
  - Tail: the very last piece (batch 7 of block 7) is fed to the PE
    directly (32 tiny accumulating matmuls), so no DVE reduce sits on the
    critical path after the last byte; its two PSUM->SBUF copies run in
    parallel on DVE and ACT, each gating only its half of block 7's
    B-matmul.
  - Squash: ACT square+row-accum -> n2 (single PSUM read), sqrt
    back-to-back on ACT in parallel with DVE 1/(1+n2), then one fused
    two-scalar DVE multiply (sbar * r * rd).
  - Store: plain HWDGE DMA of the [8, 64] v rows.  (A prepared-SWDGE
    scatter + trigger_dma store would cut another ~1.2us of issue latency,
    but this walrus build cannot codegen the trigger/reload ISA ops.)  The
    host broadcasts the (mathematically degenerate) out-caps axis.

Chunk schedule: 4-batch chunks for blocks 0-4, 2-batch for blocks 5-6,
single-batch for block 7 — tapered so the DVE reduce pipeline carries no
backlog into the tail, while keeping trailing DMA count low enough that the
~625ns/DMA HWDGE prep stage stays ahead of the transfers.

Preamble fix: Bass unconditionally emits 4 const-pool memsets on Pool BEFORE
the all-engine start barrier; only const-float32-0.0 is read here (the ACT
sqrt's bias operand), so a module pass moves the unreferenced ones after the
barrier (-250ns off the critical head).
"""

import json

import numpy as np

import concourse.bass as bass
import concourse.tile as tile
from concourse import mybir
from concourse.bass_utils import run_bass_kernel_spmd

N_CORES = 8
BS = 64
BPC = BS // N_CORES  # 8 batches per core
NCH = 1024           # num_shared * in_dim channels
HW = 256             # 16*16 spatial
NS = 32              # num shared groups
IN_DIM = 32
OUT_DIM = 64
NUM_OUT = 64
NBLK = 8             # channel blocks of 128
F32 = mybir.dt.float32
BF16 = mybir.dt.bfloat16
I16 = mybir.dt.int16

# stash of the last run's BassKernelResults for test harnesses
LAST_RESULTS = None
_NC_CACHE = None


def _split_multi_waits(bir: bytes) -> bytes:
    """The walrus build in this toolchain only accepts a single sync-wait
    command per instruction; Tile freely attaches several (most notably the
    kernel-tail drain, which waits on every outstanding semaphore).  Rewrite
    the BIR so any instruction with N>1 waits is preceded by N-1 single-wait
    NoOps on the same engine — semantically identical (the engine stalls at
    the nops), and acceptable to this codegen."""
    j = json.loads(bir)
    ctr = [0]

    def fix_block(b):
        new = []
        for inst in b.get("instructions", []):
            si = inst.get("sync_info")
            if si:
                waits = si.get("on_wait") or []
                if len(waits) > 1:
                    for w in waits[:-1]:
                        ctr[0] += 1
                        new.append({
                            "debug": inst.get("debug", 0),
                            "engine": inst["engine"],
                            "ins": [],
                            "name": f"W-{ctr[0]}",
                            "opcode": "NoOp",
                            "outs": [],
                            "sync_info": {"on_update": [], "on_wait": [w]},
                        })
                    si["on_wait"] = [waits[-1]]
            new.append(inst)
        b["instructions"] = new
        for sb in b.get("blocks", []):
            fix_block(sb)

    for f in j.get("functions", []):
        for b in f.get("blocks", []):
            fix_block(b)
    return json.dumps(j).encode()


def _relocate_const_memsets(nc) -> None:
    """Move the 4 Bass-preamble const-pool memsets (const-float32-0.0 etc.,
    emitted on Pool BEFORE the all-engine start barrier) to just after the
    preamble.  Nothing in this module reads those tensors (verified: no
    instruction 'ins' references a const-* memref), but they serialize with
    Pool's barrier arrival and delay the stream start by ~360ns."""
    # collect const tensors actually read by some instruction (e.g. the ACT
    # sqrt's bias operand reads const-float32-0.0) — those memsets must stay
    # before the barrier
    used = set()

    def scan(b):
        for i in b.instructions:
            for a in list(getattr(i, "ins", []) or []):
                ref = getattr(a, "memref", "")
                if isinstance(ref, str) and ref.startswith("const-"):
                    used.add(ref)
        for sb in getattr(b, "blocks", []):
            scan(sb)

    for b in nc.m.functions[0].blocks:
        scan(b)

    insts = list(nc.m.functions[0].blocks[0].instructions)
    const_ms = []
    for i in insts:
        if type(i).__name__ == "InstMemset" and i.outs:
            ref = getattr(i.outs[0], "memref", "")
            if isinstance(ref, str) and ref.startswith("const-") and \
                    ref not in used:
                const_ms.append(i)
    if not const_ms:
        return
    preamble_ops = {
        "InstCall", "InstRegisterMove", "InstMemset", "InstDrain",
        "InstEventSemaphore",
    }
    cut = None
    for idx, i in enumerate(insts):
        if type(i).__name__ not in preamble_ops:
            cut = idx
            break
    if cut is None:
        return
    kept = [i for i in insts[:cut] if i not in const_ms]
    nc.m.functions[0].blocks[0].instructions = (
        kept + const_ms + insts[cut:]
    )


def _build():
    nc = bass.Bass()
    x = nc.dram_tensor("x", [BPC, NCH, HW], F32, kind="ExternalInput")
    # wt3[(g*32+i), j, o] = W[4j+g, o, i] / 64, bf16 (host-packed)
    wt3 = nc.dram_tensor("wt3", [128, NBLK, OUT_DIM], BF16, kind="ExternalInput")
    # the out-caps axis of v is mathematically degenerate (identical for all
    # o) — the device emits only the unique [b, d] rows; the host unshard
    # step broadcasts to the full [b, o, d] shape.
    out = nc.dram_tensor("out", [BPC, OUT_DIM], F32, kind="ExternalOutput")

    with tile.TileContext(nc) as tc:
        with (
            tc.tile_pool(name="consts", bufs=1) as consts,
            tc.tile_pool(name="xp", bufs=29) as xp,
            tc.tile_pool(name="tp", bufs=3) as tp,
            tc.tile_pool(name="usb", bufs=3) as usb,
            tc.tile_pool(name="ep", bufs=1) as ep,
            tc.tile_pool(name="pp", bufs=3, space="PSUM") as pp,
            tc.tile_pool(name="sp", bufs=1, space="PSUM") as spp,
        ):
            # ---- constants & early stores -------------------------------
            wt3_sb = consts.tile([128, NBLK, OUT_DIM], BF16)
            nc.scalar.dma_start(out=wt3_sb, in_=wt3[:])

            # group-selector matrix sel[c, g] = (c // 32 == g)
            sel = consts.tile([128, 4], F32)
            nc.vector.memset(sel, 0.0)
            for g in range(4):
                nc.vector.memset(sel[32 * g:32 * (g + 1), g:g + 1], 1.0)

            vrow = ep.tile([BPC, OUT_DIM], F32)

            sbar = spp.tile([BPC, OUT_DIM], F32)

            # ---- the x stream -------------------------------------------
            # xv[p, j, b, m] = x[b, j*128 + p, m]
            xv = x.rearrange("b (j p) m -> p j b m", p=128)

            # taper chunk sizes toward the stream end so the DVE reduce
            # pipeline (327ns per batch vs 364ns arrival per batch) carries
            # no backlog into the tail
            chunks = []
            for j in range(5):
                chunks += [(j, 0, 4), (j, 4, 8)]
            for j in (5, 6):
                chunks += [(j, 0, 2), (j, 2, 4), (j, 4, 6), (j, 6, 8)]
            chunks += [(7, b, b + 1) for b in range(7)]

            tks = {}

            def a2(j, b, tk):
                # PE out base partition must be in {0, 32, 64}: stack the 4
                # groups as two 64-partition PSUM tiles (offsets 0/32 each)
                for g in range(4):
                    u3 = u3s[j][g // 2]
                    nc.tensor.matmul(
                        out=u3[32 * (g % 2):32 * (g % 2 + 1), b:b + 1],
                        lhsT=tk[:, b, :],
                        rhs=sel[:, g:g + 1],
                        start=True,
                        stop=True,
                        skip_group_check=True,
                    )

            def bstage(j, lhsT):
                nc.tensor.matmul(
                    out=sbar,
                    lhsT=lhsT,
                    rhs=wt3_sb[:, j, :],
                    start=(j == 0),
                    stop=False,
                    skip_group_check=True,
                )

            u3s = {}
            u3sbs = {}
            done_blocks = []

            def flush_done_blocks():
                # copies+B for a completed block are emitted AFTER the next
                # chunk's reduce: keeps the DVE queue head from stalling on
                # the +173ns PE matmul completion latency.  Mid-stream copies
                # run on the otherwise-idle ACT engine to keep DVE lean.
                for j in done_blocks:
                    u3sbs[j] = usb.tile([128, BPC], BF16, tag="u3sb",
                                        name=f"u3sb_{j}")
                    nc.scalar.copy(out=u3sbs[j][0:64, :], in_=u3s[j][0])
                    nc.scalar.copy(out=u3sbs[j][64:128, :], in_=u3s[j][1])
                    bstage(j, u3sbs[j][:, :])
                done_blocks.clear()

            for (j, b0, b1) in chunks:
                nb = b1 - b0
                xt = xp.tile([128, nb, HW], F32, tag="xt", name=f"xt_{j}_{b0}")
                nc.sync.dma_start(out=xt, in_=xv[:, j, b0:b1, :])
                if j not in tks:
                    tks[j] = tp.tile([128, BPC, IN_DIM], F32, tag="tk",
                                     name=f"tk_{j}")
                    u3s[j] = (
                        pp.tile([64, BPC], F32, tag="u3l", name=f"u3l_{j}"),
                        pp.tile([64, BPC], F32, tag="u3h", name=f"u3h_{j}"),
                    )
                tk = tks[j]
                # spatial m = k*32 + i ; reduce over the 8 k-repeats for all
                # batches of this chunk in one DVE op
                nc.vector.reduce_sum(
                    out=tk[:, b0:b1, :],
                    in_=xt.rearrange("p b (k i) -> p b i k", i=IN_DIM),
                    axis=mybir.AxisListType.X,
                )
                flush_done_blocks()
                for b in range(b0, b1):
                    a2(j, b, tk)
                if b1 == BPC and j < NBLK - 1:
                    done_blocks.append(j)

            # ---- tail: batch 7 of block 7, PE-direct --------------------
            # the last piece (batch 7 of block 7) arrives as two 64KB
            # halves and feeds the PE directly: the channel-group contraction
            # and the k-sum happen in one PSUM accumulation chain, so the
            # post-last-byte path has no DVE reduce on it.
            xlast = xp.tile([128, 1, HW], F32, tag="xh", name="x_last")
            nc.sync.dma_start(out=xlast, in_=xv[:, 7, 7:8, :])
            for k in range(8):
                for g in range(4):
                    u3 = u3s[7][g // 2]
                    nc.tensor.matmul(
                        out=u3[32 * (g % 2):32 * (g % 2 + 1), 7:8],
                        lhsT=xlast[:, 0, 32 * k:32 * (k + 1)],
                        rhs=sel[:, g:g + 1],
                        start=(k == 0),
                        stop=(k == 7),
                        skip_group_check=True,
                    )
            # tail copies run in parallel (DVE + ACT); each half of block
            # 7's B-matmul waits only on its own copy, so the contraction
            # overlaps the copy chain
            u3sb7 = usb.tile([128, BPC], BF16, tag="u3sb", name="u3sb_7")
            nc.vector.tensor_copy(out=u3sb7[0:64, :], in_=u3s[7][0])
            nc.scalar.copy(out=u3sb7[64:128, :], in_=u3s[7][1])
            nc.tensor.matmul(
                out=sbar, lhsT=u3sb7[0:64, :], rhs=wt3_sb[0:64, 7, :],
                start=False, stop=False, skip_group_check=True,
            )
            nc.tensor.matmul(
                out=sbar, lhsT=u3sb7[64:128, :], rhs=wt3_sb[64:128, 7, :],
                start=False, stop=True, skip_group_check=True,
            )

            # ---- squash: v = sbar * sqrt(n2)/(1+n2), n2 = |sbar|^2 ------
            # ACT square+row-accum (single PSUM read — walrus only allows one
            # PSUM input per instruction), then sqrt back-to-back on ACT (no
            # cross-engine hop for n2); the 1/(1+n2) branch runs on DVE in
            # parallel.
            sq = ep.tile([BPC, OUT_DIM], F32)
            n2 = ep.tile([BPC, 1], F32)
            nc.scalar.activation(
                out=sq, in_=sbar,
                func=mybir.ActivationFunctionType.Square,
                accum_out=n2,
            )
            r = ep.tile([BPC, 1], F32)
            nc.scalar.sqrt(out=r, in_=n2)
            d = ep.tile([BPC, 1], F32)
            nc.vector.tensor_scalar_add(out=d, in0=n2, scalar1=1.0)
            rd = ep.tile([BPC, 1], F32)
            nc.vector.reciprocal(out=rd, in_=d)
            # vrow = (sbar * r) * rd in ONE fused two-scalar DVE op
            nc.vector.tensor_scalar(
                out=vrow, in0=sbar,
                scalar1=r, scalar2=rd,
                op0=mybir.AluOpType.mult, op1=mybir.AluOpType.mult,
            )
            nc.sync.dma_start(out=out[:], in_=vrow)

    _relocate_const_memsets(nc)
    # every compile path (native walrus + bass2jax/axon) serializes via
    # to_json_bytes — splice the single-wait rewrite in there
    orig_to_json = nc.to_json_bytes
    nc.to_json_bytes = lambda: _split_multi_waits(orig_to_json())
    return nc


def _pack_wt3(W: np.ndarray) -> np.ndarray:
    """wt3[g*32+i, j, o] = W[4j+g, o, i] / 64, bf16."""
    import ml_dtypes

    t = W.reshape(NBLK, 4, OUT_DIM, IN_DIM)          # [j, g, o, i]
    t = t.transpose(1, 3, 0, 2)                      # [g, i, j, o]
    t = t.reshape(128, NBLK, OUT_DIM) * np.float32(1.0 / 64.0)
    return np.ascontiguousarray(t.astype(ml_dtypes.bfloat16))


def kernel(x: np.ndarray, W: np.ndarray, trace: bool = False) -> np.ndarray:
    global LAST_RESULTS, _NC_CACHE
    x = np.ascontiguousarray(np.asarray(x, dtype=np.float32)).reshape(BS, NCH, HW)
    W = np.asarray(W, dtype=np.float32)
    wt3 = _pack_wt3(W)

    if _NC_CACHE is None:
        _NC_CACHE = _build()
    nc = _NC_CACHE
    in_maps = [
        {"x": np.ascontiguousarray(x[c * BPC:(c + 1) * BPC]), "wt3": wt3}
        for c in range(N_CORES)
    ]
    res = run_bass_kernel_spmd(nc, in_maps, core_ids=list(range(N_CORES)), trace=trace)
    LAST_RESULTS = res
    rows = np.concatenate([r["out"] for r in res.results], axis=0)  # [64, 64]
    # unshard: materialize the degenerate out-caps axis (v is identical for
    # every o — see the module docstring)
    return np.ascontiguousarray(
        np.broadcast_to(rows[:, None, :], (BS, NUM_OUT, OUT_DIM))
    )


# revision 28
# speedup vs baseline: 1.0094x; 1.0094x over previous
"""Bass/Trainium2 kernel for nn_CapLayer (dynamic-routing capsule layer).

Key algebraic identity (holds for ANY x, W — verified against the reference):
the routing logits b start at zero; softmax over the out-caps axis of an
o-constant tensor is uniform (1/NUM_OUT); with uniform c the squashed v is
identical for every out-cap o, which makes delta_b = <pred, v> o-constant as
well, so b stays o-constant through every routing iteration and the softmax
stays uniform forever.  Hence:

    sbar[b, :] = (1/64) * sum_n pred[b, n, :]
               = (1/64) * sum_{s,i} (sum_p u[b,s,p,i]) * W[s,:,i]
    v[b, o, :] = sbar[b,:] * (|sbar| / (1 + |sbar|^2))     for all o.

So the kernel is: a full reduction of x over the per-group spatial axis
(memory bound — must read all of x exactly once at ~2.9us/MB on the single
shared DMA-engine resource), a tiny matmul with a rearranged W, a squash,
and a broadcast store.  Data-parallel over batch across 8 cores.

On-chip dataflow per core (8 batches, 8 channel-blocks of 128):
  - DMA chunks [128c, nb, 256m] of x stream in back-to-back (~23.3us total).
  - DVE: segmented reduce over the 8 spatial repeats: [128c, nb, 256] ->
    tk[128c, nb, 32i].
  - PE (A2): per (batch, group-of-4): lhsT=tk[:, b, :], rhs=sel[:, g] ->
    u3[j][32g:32g+32, b] in PSUM, i.e. u3 lands directly in the stacked
    [(g i), b] layout the B-stage needs.
  - DVE: one [128, 8] f32->bf16 copy per block -> u3sb[j].
  - PE (B): ONE bf16 matmul per block: sbar[8b, 64o] += u3sb[j]^T @ wt3[j]
    (wt3[(g i), j, o] = W[4j+g, o, i]/64, pre-scaled+packed bf16 on host).
  - Tail: the very last piece (batch 7 of block 7) is fed to the PE
    directly (32 tiny accumulating matmuls), so no DVE reduce sits on the
    critical path after the last byte; its two PSUM->SBUF copies run in
    parallel on DVE and ACT, each gating only its half of block 7's
    B-matmul.
  - Squash: ACT square+row-accum -> n2 (single PSUM read), sqrt
    back-to-back on ACT in parallel with DVE 1/(1+n2), then one fused
    two-scalar DVE multiply (sbar * r * rd).
  - Store: plain HWDGE DMA of the [8, 64] v rows.  (A prepared-SWDGE
    scatter + trigger_dma store would cut another ~1.2us of issue latency,
    but this walrus build cannot codegen the trigger/reload ISA ops.)  The
    host broadcasts the (mathematically degenerate) out-caps axis.

Chunk schedule: 4-batch chunks for blocks 0-4, 2-batch for blocks 5-6,
single-batch for block 7 — tapered so the DVE reduce pipeline carries no
backlog into the tail, while keeping trailing DMA count low enough that the
~625ns/DMA HWDGE prep stage stays ahead of the transfers.

Preamble fix: Bass unconditionally emits 4 const-pool memsets on Pool BEFORE
the all-engine start barrier; only const-float32-0.0 is read here (the ACT
sqrt's bias operand), so a module pass moves the unreferenced ones after the
barrier (-250ns off the critical head).
"""

import json

import numpy as np

import concourse.bass as bass
import concourse.tile as tile
from concourse import mybir
from concourse.bass_utils import run_bass_kernel_spmd

N_CORES = 8
BS = 64
BPC = BS // N_CORES  # 8 batches per core
NCH = 1024           # num_shared * in_dim channels
HW = 256             # 16*16 spatial
NS = 32              # num shared groups
IN_DIM = 32
OUT_DIM = 64
NUM_OUT = 64
NBLK = 8             # channel blocks of 128
F32 = mybir.dt.float32
BF16 = mybir.dt.bfloat16
I16 = mybir.dt.int16

# stash of the last run's BassKernelResults for test harnesses
LAST_RESULTS = None
_NC_CACHE = None


def _split_multi_waits(bir: bytes) -> bytes:
    """The walrus build in this toolchain only accepts a single sync-wait
    command per instruction; Tile freely attaches several (most notably the
    kernel-tail drain, which waits on every outstanding semaphore).  Rewrite
    the BIR so any instruction with N>1 waits is preceded by N-1 single-wait
    NoOps on the same engine — semantically identical (the engine stalls at
    the nops), and acceptable to this codegen."""
    j = json.loads(bir)
    ctr = [0]

    def fix_block(b):
        new = []
        for inst in b.get("instructions", []):
            si = inst.get("sync_info")
            if si:
                waits = si.get("on_wait") or []
                if len(waits) > 1:
                    for w in waits[:-1]:
                        ctr[0] += 1
                        new.append({
                            "debug": inst.get("debug", 0),
                            "engine": inst["engine"],
                            "ins": [],
                            "name": f"W-{ctr[0]}",
                            "opcode": "NoOp",
                            "outs": [],
                            "sync_info": {"on_update": [], "on_wait": [w]},
                        })
                    si["on_wait"] = [waits[-1]]
            new.append(inst)
        b["instructions"] = new
        for sb in b.get("blocks", []):
            fix_block(sb)

    for f in j.get("functions", []):
        for b in f.get("blocks", []):
            fix_block(b)
    return json.dumps(j).encode()


def _relocate_const_memsets(nc) -> None:
    """Move the 4 Bass-preamble const-pool memsets (const-float32-0.0 etc.,
    emitted on Pool BEFORE the all-engine start barrier) to just after the
    preamble.  Nothing in this module reads those tensors (verified: no
    instruction 'ins' references a const-* memref), but they serialize with
    Pool's barrier arrival and delay the stream start by ~360ns."""
    # collect const tensors actually read by some instruction (e.g. the ACT
    # sqrt's bias operand reads const-float32-0.0) — those memsets must stay
    # before the barrier
    used = set()

    def scan(b):
        for i in b.instructions:
            for a in list(getattr(i, "ins", []) or []):
                ref = getattr(a, "memref", "")
                if isinstance(ref, str) and ref.startswith("const-"):
                    used.add(ref)
        for sb in getattr(b, "blocks", []):
            scan(sb)

    for b in nc.m.functions[0].blocks:
        scan(b)

    insts = list(nc.m.functions[0].blocks[0].instructions)
    const_ms = []
    for i in insts:
        if type(i).__name__ == "InstMemset" and i.outs:
            ref = getattr(i.outs[0], "memref", "")
            if isinstance(ref, str) and ref.startswith("const-") and \
                    ref not in used:
                const_ms.append(i)
        # engine-preamble register setup is engine-local (sem-base regs
        # consumed only by that engine's own body instructions), so every
        # non-SP engine's RegisterMoves can also run after the barrier; SP's
        # stay put because SP issues the first x DMA right at release.  This
        # moves the barrier-gather laggard from PE (~551ns) to SP (~275ns).
        if type(i).__name__ == "InstRegisterMove" and \
                i.engine != mybir.EngineType.SP:
            const_ms.append(i)
    if not const_ms:
        return
    preamble_ops = {
        "InstCall", "InstRegisterMove", "InstMemset", "InstDrain",
        "InstEventSemaphore",
    }
    cut = None
    for idx, i in enumerate(insts):
        if type(i).__name__ not in preamble_ops:
            cut = idx
            break
    if cut is None:
        return
    kept = [i for i in insts[:cut] if i not in const_ms]
    nc.m.functions[0].blocks[0].instructions = (
        kept + const_ms + insts[cut:]
    )


def _build():
    nc = bass.Bass()
    x = nc.dram_tensor("x", [BPC, NCH, HW], F32, kind="ExternalInput")
    # wt3[(g*32+i), j, o] = W[4j+g, o, i] / 64, bf16 (host-packed)
    wt3 = nc.dram_tensor("wt3", [128, NBLK, OUT_DIM], BF16, kind="ExternalInput")
    # the out-caps axis of v is mathematically degenerate (identical for all
    # o) — the device emits only the unique [b, d] rows; the host unshard
    # step broadcasts to the full [b, o, d] shape.
    out = nc.dram_tensor("out", [BPC, OUT_DIM], F32, kind="ExternalOutput")

    with tile.TileContext(nc) as tc:
        with (
            tc.tile_pool(name="consts", bufs=1) as consts,
            tc.tile_pool(name="xp", bufs=29) as xp,
            tc.tile_pool(name="tp", bufs=3) as tp,
            tc.tile_pool(name="usb", bufs=3) as usb,
            tc.tile_pool(name="ep", bufs=1) as ep,
            tc.tile_pool(name="pp", bufs=3, space="PSUM") as pp,
            tc.tile_pool(name="sp", bufs=1, space="PSUM") as spp,
        ):
            # ---- constants & early stores -------------------------------
            wt3_sb = consts.tile([128, NBLK, OUT_DIM], BF16)
            nc.scalar.dma_start(out=wt3_sb, in_=wt3[:])

            # group-selector matrix sel[c, g] = (c // 32 == g)
            sel = consts.tile([128, 4], F32)
            nc.vector.memset(sel, 0.0)
            for g in range(4):
                nc.vector.memset(sel[32 * g:32 * (g + 1), g:g + 1], 1.0)

            vrow = ep.tile([BPC, OUT_DIM], F32)

            sbar = spp.tile([BPC, OUT_DIM], F32)

            # ---- the x stream -------------------------------------------
            # xv[p, j, b, m] = x[b, j*128 + p, m]
            xv = x.rearrange("b (j p) m -> p j b m", p=128)

            # taper chunk sizes toward the stream end so the DVE reduce
            # pipeline (327ns per batch vs 364ns arrival per batch) carries
            # no backlog into the tail
            chunks = []
            for j in range(5):
                chunks += [(j, 0, 4), (j, 4, 8)]
            for j in (5, 6):
                chunks += [(j, 0, 2), (j, 2, 4), (j, 4, 6), (j, 6, 8)]
            chunks += [(7, b, b + 1) for b in range(7)]

            tks = {}

            def a2(j, b, tk):
                # PE out base partition must be in {0, 32, 64}: stack the 4
                # groups as two 64-partition PSUM tiles (offsets 0/32 each)
                for g in range(4):
                    u3 = u3s[j][g // 2]
                    nc.tensor.matmul(
                        out=u3[32 * (g % 2):32 * (g % 2 + 1), b:b + 1],
                        lhsT=tk[:, b, :],
                        rhs=sel[:, g:g + 1],
                        start=True,
                        stop=True,
                        skip_group_check=True,
                    )

            def bstage(j, lhsT):
                nc.tensor.matmul(
                    out=sbar,
                    lhsT=lhsT,
                    rhs=wt3_sb[:, j, :],
                    start=(j == 0),
                    stop=False,
                    skip_group_check=True,
                )

            u3s = {}
            u3sbs = {}
            done_blocks = []

            def flush_done_blocks():
                # copies+B for a completed block are emitted AFTER the next
                # chunk's reduce: keeps the DVE queue head from stalling on
                # the +173ns PE matmul completion latency.  Mid-stream copies
                # run on the otherwise-idle ACT engine to keep DVE lean.
                for j in done_blocks:
                    u3sbs[j] = usb.tile([128, BPC], BF16, tag="u3sb",
                                        name=f"u3sb_{j}")
                    nc.scalar.copy(out=u3sbs[j][0:64, :], in_=u3s[j][0])
                    nc.scalar.copy(out=u3sbs[j][64:128, :], in_=u3s[j][1])
                    bstage(j, u3sbs[j][:, :])
                done_blocks.clear()

            for (j, b0, b1) in chunks:
                nb = b1 - b0
                xt = xp.tile([128, nb, HW], F32, tag="xt", name=f"xt_{j}_{b0}")
                nc.sync.dma_start(out=xt, in_=xv[:, j, b0:b1, :])
                if j not in tks:
                    tks[j] = tp.tile([128, BPC, IN_DIM], F32, tag="tk",
                                     name=f"tk_{j}")
                    u3s[j] = (
                        pp.tile([64, BPC], F32, tag="u3l", name=f"u3l_{j}"),
                        pp.tile([64, BPC], F32, tag="u3h", name=f"u3h_{j}"),
                    )
                tk = tks[j]
                # spatial m = k*32 + i ; reduce over the 8 k-repeats for all
                # batches of this chunk in one DVE op
                nc.vector.reduce_sum(
                    out=tk[:, b0:b1, :],
                    in_=xt.rearrange("p b (k i) -> p b i k", i=IN_DIM),
                    axis=mybir.AxisListType.X,
                )
                flush_done_blocks()
                for b in range(b0, b1):
                    a2(j, b, tk)
                if b1 == BPC and j < NBLK - 1:
                    done_blocks.append(j)

            # ---- tail: batch 7 of block 7, PE-direct --------------------
            # the last piece (batch 7 of block 7) arrives as two 64KB
            # halves and feeds the PE directly: the channel-group contraction
            # and the k-sum happen in one PSUM accumulation chain, so the
            # post-last-byte path has no DVE reduce on it.
            xlast = xp.tile([128, 1, HW], F32, tag="xh", name="x_last")
            nc.sync.dma_start(out=xlast, in_=xv[:, 7, 7:8, :])
            for k in range(8):
                for g in range(4):
                    u3 = u3s[7][g // 2]
                    nc.tensor.matmul(
                        out=u3[32 * (g % 2):32 * (g % 2 + 1), 7:8],
                        lhsT=xlast[:, 0, 32 * k:32 * (k + 1)],
                        rhs=sel[:, g:g + 1],
                        start=(k == 0),
                        stop=(k == 7),
                        skip_group_check=True,
                    )
            # tail copies run in parallel (DVE + ACT); each half of block
            # 7's B-matmul waits only on its own copy, so the contraction
            # overlaps the copy chain
            u3sb7 = usb.tile([128, BPC], BF16, tag="u3sb", name="u3sb_7")
            nc.vector.tensor_copy(out=u3sb7[0:64, :], in_=u3s[7][0])
            nc.scalar.copy(out=u3sb7[64:128, :], in_=u3s[7][1])
            nc.tensor.matmul(
                out=sbar, lhsT=u3sb7[0:64, :], rhs=wt3_sb[0:64, 7, :],
                start=False, stop=False, skip_group_check=True,
            )
            nc.tensor.matmul(
                out=sbar, lhsT=u3sb7[64:128, :], rhs=wt3_sb[64:128, 7, :],
                start=False, stop=True, skip_group_check=True,
            )

            # ---- squash: v = sbar * sqrt(n2)/(1+n2), n2 = |sbar|^2 ------
            # ACT square+row-accum (single PSUM read — walrus only allows one
            # PSUM input per instruction), then sqrt back-to-back on ACT (no
            # cross-engine hop for n2); the 1/(1+n2) branch runs on DVE in
            # parallel.
            sq = ep.tile([BPC, OUT_DIM], F32)
            n2 = ep.tile([BPC, 1], F32)
            nc.scalar.activation(
                out=sq, in_=sbar,
                func=mybir.ActivationFunctionType.Square,
                accum_out=n2,
            )
            r = ep.tile([BPC, 1], F32)
            nc.scalar.sqrt(out=r, in_=n2)
            d = ep.tile([BPC, 1], F32)
            nc.vector.tensor_scalar_add(out=d, in0=n2, scalar1=1.0)
            rd = ep.tile([BPC, 1], F32)
            nc.vector.reciprocal(out=rd, in_=d)
            # vrow = (sbar * r) * rd in ONE fused two-scalar DVE op
            nc.vector.tensor_scalar(
                out=vrow, in0=sbar,
                scalar1=r, scalar2=rd,
                op0=mybir.AluOpType.mult, op1=mybir.AluOpType.mult,
            )
            nc.sync.dma_start(out=out[:], in_=vrow)

    _relocate_const_memsets(nc)
    # every compile path (native walrus + bass2jax/axon) serializes via
    # to_json_bytes — splice the single-wait rewrite in there
    orig_to_json = nc.to_json_bytes
    nc.to_json_bytes = lambda: _split_multi_waits(orig_to_json())
    return nc


def _pack_wt3(W: np.ndarray) -> np.ndarray:
    """wt3[g*32+i, j, o] = W[4j+g, o, i] / 64, bf16."""
    import ml_dtypes

    t = W.reshape(NBLK, 4, OUT_DIM, IN_DIM)          # [j, g, o, i]
    t = t.transpose(1, 3, 0, 2)                      # [g, i, j, o]
    t = t.reshape(128, NBLK, OUT_DIM) * np.float32(1.0 / 64.0)
    return np.ascontiguousarray(t.astype(ml_dtypes.bfloat16))


def kernel(x: np.ndarray, W: np.ndarray, trace: bool = False) -> np.ndarray:
    global LAST_RESULTS, _NC_CACHE
    x = np.ascontiguousarray(np.asarray(x, dtype=np.float32)).reshape(BS, NCH, HW)
    W = np.asarray(W, dtype=np.float32)
    wt3 = _pack_wt3(W)

    if _NC_CACHE is None:
        _NC_CACHE = _build()
    nc = _NC_CACHE
    in_maps = [
        {"x": np.ascontiguousarray(x[c * BPC:(c + 1) * BPC]), "wt3": wt3}
        for c in range(N_CORES)
    ]
    res = run_bass_kernel_spmd(nc, in_maps, core_ids=list(range(N_CORES)), trace=trace)
    LAST_RESULTS = res
    rows = np.concatenate([r["out"] for r in res.results], axis=0)  # [64, 64]
    # unshard: materialize the degenerate out-caps axis (v is identical for
    # every o — see the module docstring)
    return np.ascontiguousarray(
        np.broadcast_to(rows[:, None, :], (BS, NUM_OUT, OUT_DIM))
    )
